# revision 9
# baseline (speedup 1.0000x reference)
"""Trainium2 Bass kernel for the eigenvalue/eigenvector loss
(nn_AV_loss): per-voxel 3x3 symmetric eigendecomposition of input and
target tensors, masked L1 of sorted eigenvalues + masked principal-axis
|cosine|, reduced to two scalars.

Self-contained: hardcodes shapes/sharding. kernel(**inputs) takes FULL
inputs and returns the full output (val_loss, vec_loss).

Sharding: fully data-parallel over B*H (2*80 = 160 -> 20 H-slices per
core); per-core partial masked sums are returned and reduced on host.

Math (per 3x3 symmetric matrix A = [[a,d,e],[d,b,f],[e,f,c]]):
  trigonometric (Smith) eigensolver:
    q = tr/3, p = sqrt(p2/6) with p2 = sum of squared deviator entries,
    r = det(A-qI)/(2 p^3) clamped to [-1,1],
    acos(r) = pi/2 - arctan(r/sqrt(1-r^2)),
    lam_max = q + 2p cos(acos(r)/3) = q + 2p sin(arctan/3 + pi/3),
    lam_min = q - 2p sin(-arctan/3 + pi/3), lam_mid = 3q - max - min.
  principal eigenvector via cross product of the first two rows of
  (A - lam_max I)  (parallel-rows failure set has measure ~0 and its
  bounded error washes out in the 512k-voxel masked mean).

Data layout per core: channel planes [128, PK] where the free dim packs
[input chunk | target chunk]; all per-matrix math runs on packed planes,
the input-vs-target stage slices the two halves.

ACT table-set phases (avoid table thrash): A (sqrt set), B (trig set),
C (sqrt set), emitted globally across chunks -> 3 table loads total.
"""

import numpy as np

import concourse.bass as bass
import concourse.tile as tile
from concourse.bacc import Bacc
from concourse import mybir
from concourse.bass_utils import run_bass_kernel_spmd

AF = mybir.ActivationFunctionType
OP = mybir.AluOpType
F32 = mybir.dt.float32

NCORES = 8
B, C, H, W, D = 2, 6, 80, 80, 80
HS = H // (NCORES // B)          # 20 h-slices per core
VPC = HS * W * D                 # 128000 voxels per core
P = 128
FV = VPC // P                    # 1000 voxel columns per partition
NCH = 4                          # chunks along the free dim
FC = FV // NCH                   # voxel cols per chunk
PK = 2 * FC                      # packed cols per chunk: [input | target]

CLAMP = 1.0 - 3e-7               # |r| clamp so 1-r^2 stays positive in f32
PI3 = float(np.pi / 3.0)


def _build():
    nc = Bacc()
    x = nc.dram_tensor("x", [C, P, NCH, PK], F32, kind="ExternalInput")
    mf = nc.dram_tensor("mf", [P, NCH, FC], F32, kind="ExternalInput")
    out = nc.dram_tensor("out", [P, 2 * NCH], F32, kind="ExternalOutput")

    with tile.TileContext(nc) as tc:
        with tc.tile_pool(name="main", bufs=1) as pool:

            def T(tag, cols=PK):          # per-chunk persisted value
                return pool.tile([P, cols], F32, tag=tag, bufs=NCH, name=tag)

            def TA():                      # phase-A transient
                return pool.tile([P, PK], F32, tag="tA", bufs=12, name="tA")

            def TB():                      # phase-B transient (packed)
                return pool.tile([P, PK], F32, tag="tB", bufs=12, name="tB")

            def TH():                      # half-width transient
                return pool.tile([P, FC], F32, tag="tH", bufs=8, name="tH")

            out_sb = pool.tile([P, 2 * NCH], F32, tag="out_sb", bufs=1)
            pi3c = pool.tile([P, 1], F32, tag="pi3c", bufs=1)
            nc.vector.memset(pi3c, PI3)
            c05 = pool.tile([P, 1], F32, tag="c05", bufs=1)
            nc.vector.memset(c05, 0.5)

            # ---- load all chunks (DMA overlaps downstream compute) ----
            chans = []
            masks = []
            for cidx in range(NCH):
                cd = {}
                # _SYM_IDX packing: a=ch0 b=ch3 c=ch5 d=ch1 e=ch2 f=ch4
                for nm, ch in (("a", 0), ("b", 3), ("c", 5),
                               ("d", 1), ("e", 2), ("f", 4)):
                    tl = T("ch_" + nm)
                    nc.sync.dma_start(out=tl, in_=x[ch, :, cidx, :])
                    cd[nm] = tl
                mt = T("mf", cols=FC)
                nc.sync.dma_start(out=mt, in_=mf[:, cidx, :])
                chans.append(cd)
                masks.append(mt)

            # ---- phase A (sqrt act-set): invariants, p, r, arg ----
            pers = []
            for cidx in range(NCH):
                ch = chans[cidx]
                a, b, c = ch["a"], ch["b"], ch["c"]
                d, e, f = ch["d"], ch["e"], ch["f"]

                sAB = TA()
                nc.vector.tensor_add(out=sAB, in0=a, in1=b)
                s3 = T("s3")
                nc.vector.tensor_add(out=s3, in0=sAB, in1=c)
                q = T("q")
                nc.vector.tensor_scalar_mul(out=q, in0=s3, scalar1=1.0 / 3.0)
                aq = TA()
                nc.vector.scalar_tensor_tensor(out=aq, in0=s3, scalar=-1.0 / 3.0,
                                               in1=a, op0=OP.mult, op1=OP.add)
                bq = TA()
                nc.vector.scalar_tensor_tensor(out=bq, in0=s3, scalar=-1.0 / 3.0,
                                               in1=b, op0=OP.mult, op1=OP.add)
                cq = TA()
                nc.vector.scalar_tensor_tensor(out=cq, in0=s3, scalar=-1.0 / 3.0,
                                               in1=c, op0=OP.mult, op1=OP.add)
                dd = T("dd")
                nc.scalar.activation(out=dd, in_=d, func=AF.Square)
                ee = TA()
                nc.scalar.activation(out=ee, in_=e, func=AF.Square)
                ff = TA()
                nc.scalar.activation(out=ff, in_=f, func=AF.Square)
                de = T("de")
                nc.gpsimd.tensor_tensor(out=de, in0=d, in1=e, op=OP.mult)
                p1a = TA()
                nc.gpsimd.tensor_tensor(out=p1a, in0=dd, in1=ee, op=OP.add)
                p1 = TA()
                nc.vector.tensor_add(out=p1, in0=p1a, in1=ff)
                aq2 = TA()
                nc.scalar.activation(out=aq2, in_=aq, func=AF.Square)
                bq2 = TA()
                nc.scalar.activation(out=bq2, in_=bq, func=AF.Square)
                cq2 = TA()
                nc.scalar.activation(out=cq2, in_=cq, func=AF.Square)
                t = TA()
                nc.gpsimd.tensor_tensor(out=t, in0=aq2, in1=bq2, op=OP.add)
                t2 = TA()
                nc.vector.tensor_add(out=t2, in0=t, in1=cq2)
                p2 = TA()
                nc.vector.scalar_tensor_tensor(out=p2, in0=p1, scalar=2.0,
                                               in1=t2, op0=OP.mult, op1=OP.add)
                p = T("p")
                nc.scalar.activation(out=p, in_=p2, func=AF.Sqrt,
                                     scale=1.0 / 6.0)
                p3 = TA()
                nc.vector.scalar_tensor_tensor(out=p3, in0=p2, scalar=1.0 / 6.0,
                                               in1=p, op0=OP.mult, op1=OP.mult)
                ip3 = TA()
                nc.vector.reciprocal_approx_fast(out=ip3, in_=p3)

                bc = TA()
                nc.vector.tensor_mul(out=bc, in0=bq, in1=cq)
                abc = TA()
                nc.vector.tensor_mul(out=abc, in0=aq, in1=bc)
                deff = TA()
                nc.vector.tensor_mul(out=deff, in0=de, in1=f)
                aff = TA()
                nc.gpsimd.tensor_tensor(out=aff, in0=aq, in1=ff, op=OP.mult)
                bee = TA()
                nc.gpsimd.tensor_tensor(out=bee, in0=bq, in1=ee, op=OP.mult)
                cdd = TA()
                nc.vector.tensor_mul(out=cdd, in0=cq, in1=dd)
                s1d = TA()
                nc.vector.scalar_tensor_tensor(out=s1d, in0=deff, scalar=2.0,
                                               in1=abc, op0=OP.mult, op1=OP.add)
                s2d = TA()
                nc.vector.tensor_add(out=s2d, in0=aff, in1=bee)
                s3d = TA()
                nc.vector.tensor_add(out=s3d, in0=s2d, in1=cdd)
                det = TA()
                nc.vector.tensor_sub(out=det, in0=s1d, in1=s3d)

                r0 = TA()
                nc.vector.scalar_tensor_tensor(out=r0, in0=det, scalar=0.5,
                                               in1=ip3, op0=OP.mult, op1=OP.mult)
                r = TA()
                nc.vector.tensor_scalar(out=r, in0=r0, scalar1=CLAMP,
                                        scalar2=-CLAMP, op0=OP.min, op1=OP.max)
                # half-angle: acos(r)/2 = pi/4 + atan((sm-sp)/(sm+sp)),
                # sp = cos(acos(r)/2) = sqrt((1+r)/2), sm = sin(...) = sqrt((1-r)/2)
                sp = TA()
                nc.scalar.activation(out=sp, in_=r, func=AF.Sqrt,
                                     scale=0.5, bias=c05)
                sm = TA()
                nc.scalar.activation(out=sm, in_=r, func=AF.Sqrt,
                                     scale=-0.5, bias=c05)
                num = TA()
                nc.vector.tensor_sub(out=num, in0=sm, in1=sp)
                den = TA()
                nc.gpsimd.tensor_tensor(out=den, in0=sm, in1=sp, op=OP.add)
                iden = TA()
                nc.vector.reciprocal_approx_fast(out=iden, in_=den)
                arg = T("arg")
                nc.vector.tensor_mul(out=arg, in0=num, in1=iden)
                pers.append(dict(s3=s3, q=q, p=p, arg=arg, dd=dd, de=de))

            # ---- phase B (trig act-set): angles, lambdas, eigvec, dots ----
            persB = []
            for cidx in range(NCH):
                ch = chans[cidx]
                pr = pers[cidx]
                a, b, d, e, f = ch["a"], ch["b"], ch["d"], ch["e"], ch["f"]
                s3, q, p, arg = pr["s3"], pr["q"], pr["p"], pr["arg"]
                dd, de = pr["dd"], pr["de"]

                at = TB()
                nc.scalar.activation(out=at, in_=arg, func=AF.Arctan)
                c1 = TB()
                nc.scalar.activation(out=c1, in_=at, func=AF.Sin,
                                     scale=-2.0 / 3.0, bias=pi3c)
                nc3 = TB()
                nc.scalar.activation(out=nc3, in_=at, func=AF.Sin,
                                     scale=2.0 / 3.0, bias=pi3c)
                pc1 = TB()
                nc.vector.tensor_mul(out=pc1, in0=p, in1=c1)
                l1 = TB()   # lam_max
                nc.vector.scalar_tensor_tensor(out=l1, in0=pc1, scalar=2.0,
                                               in1=q, op0=OP.mult, op1=OP.add)
                pc3 = TB()
                nc.vector.tensor_mul(out=pc3, in0=p, in1=nc3)
                l3 = TB()   # lam_min
                nc.vector.scalar_tensor_tensor(out=l3, in0=pc3, scalar=-2.0,
                                               in1=q, op0=OP.mult, op1=OP.add)
                sl = TB()
                nc.gpsimd.tensor_tensor(out=sl, in0=l1, in1=l3, op=OP.add)
                l2 = TB()   # lam_mid
                nc.vector.tensor_sub(out=l2, in0=s3, in1=sl)

                # eigvec: cross(rows 0,1) of (A - l1*I)
                a1 = TB()
                nc.vector.tensor_sub(out=a1, in0=a, in1=l1)
                b1 = TB()
                nc.vector.tensor_sub(out=b1, in0=b, in1=l1)
                m1 = TB()
                nc.gpsimd.tensor_tensor(out=m1, in0=d, in1=f, op=OP.mult)
                m2 = TB()
                nc.vector.tensor_mul(out=m2, in0=e, in1=b1)
                w1 = TB()
                nc.vector.tensor_sub(out=w1, in0=m1, in1=m2)
                m4 = TB()
                nc.vector.tensor_mul(out=m4, in0=a1, in1=f)
                w2 = TB()
                nc.vector.tensor_sub(out=w2, in0=de, in1=m4)
                m5 = TB()
                nc.vector.tensor_mul(out=m5, in0=a1, in1=b1)
                w3 = TB()
                nc.vector.tensor_sub(out=w3, in0=m5, in1=dd)

                # squared norms (packed; Square is in every act set)
                sw1 = TB()
                nc.scalar.activation(out=sw1, in_=w1, func=AF.Square)
                sw2 = TB()
                nc.scalar.activation(out=sw2, in_=w2, func=AF.Square)
                sw3 = TB()
                nc.scalar.activation(out=sw3, in_=w3, func=AF.Square)
                n12 = TB()
                nc.gpsimd.tensor_tensor(out=n12, in0=sw1, in1=sw2, op=OP.add)
                nrm = TB()
                nc.vector.tensor_add(out=nrm, in0=n12, in1=sw3)

                def IH(tl):
                    return tl[:, 0:FC]

                def THF(tl):
                    return tl[:, FC:PK]

                nn = TH()
                nc.vector.tensor_mul(out=nn, in0=IH(nrm), in1=THF(nrm))
                inn = T("inn", cols=FC)
                nc.vector.reciprocal_approx_fast(out=inn, in_=nn)

                d1 = TH()
                nc.vector.tensor_mul(out=d1, in0=IH(w1), in1=THF(w1))
                d2 = TH()
                nc.vector.tensor_mul(out=d2, in0=IH(w2), in1=THF(w2))
                d3 = TH()
                nc.vector.tensor_mul(out=d3, in0=IH(w3), in1=THF(w3))
                d12 = TH()
                nc.gpsimd.tensor_tensor(out=d12, in0=d1, in1=d2, op=OP.add)
                dotv = TH()
                nc.vector.tensor_add(out=dotv, in0=d12, in1=d3)
                adot = T("adot", cols=FC)
                nc.scalar.activation(out=adot, in_=dotv, func=AF.Abs)

                # eigenvalue L1 (halves): write the three diffs interleaved
                # into [P, FC, 3], then one abs-sum reduce over the inner axis
                dl = pool.tile([P, FC, 3], F32, tag="dl", bufs=2, name="dl")
                nc.vector.tensor_sub(out=dl[:, :, 0], in0=IH(l1), in1=THF(l1))
                nc.vector.tensor_sub(out=dl[:, :, 1], in0=IH(l2), in1=THF(l2))
                nc.vector.tensor_sub(out=dl[:, :, 2], in0=IH(l3), in1=THF(l3))
                sld = T("sld", cols=FC)
                nc.vector.tensor_reduce(out=sld, in_=dl, axis=mybir.AxisListType.X,
                                        op=OP.add, apply_absolute_value=True)
                persB.append(dict(inn=inn, adot=adot, sld=sld))

            # ---- phase C (sqrt act-set): normalize + masked reductions ----
            for cidx in range(NCH):
                pb = persB[cidx]
                mt = masks[cidx]
                rn = TH()
                nc.scalar.activation(out=rn, in_=pb["inn"], func=AF.Sqrt)
                dotn = TH()
                nc.vector.tensor_mul(out=dotn, in0=pb["adot"], in1=rn)
                junk = TH()
                nc.vector.scalar_tensor_tensor(
                    out=junk, in0=pb["sld"], scalar=1.0, in1=mt,
                    op0=OP.mult, op1=OP.mult,
                    accum_out=out_sb[:, 2 * cidx:2 * cidx + 1])
                junk2 = TH()
                nc.vector.scalar_tensor_tensor(
                    out=junk2, in0=dotn, scalar=1.0, in1=mt,
                    op0=OP.mult, op1=OP.mult,
                    accum_out=out_sb[:, 2 * cidx + 1:2 * cidx + 2])

            nc.sync.dma_start(out=out[:, :], in_=out_sb)
    nc.finalize()
    return nc


_NC = None


def _get_nc():
    global _NC
    if _NC is None:
        _NC = _build()
    return _NC


def _shard_inputs(input_data, target, mask):
    """Full inputs -> per-core in_maps (host-side pack)."""
    x = np.ascontiguousarray(input_data, dtype=np.float32)
    t = np.ascontiguousarray(target, dtype=np.float32)
    m = np.asarray(mask)
    in_maps = []
    for k in range(NCORES):
        bidx = k // (NCORES // B)
        h0 = HS * (k % (NCORES // B))
        xs = x[bidx, :, h0:h0 + HS].reshape(C, P, NCH, FC)
        ts = t[bidx, :, h0:h0 + HS].reshape(C, P, NCH, FC)
        xc = np.concatenate([xs, ts], axis=-1)          # [C,P,NCH,PK]
        mc = (m[bidx, 0, 0, h0:h0 + HS].reshape(P, NCH, FC) == 1)
        in_maps.append({
            "x": np.ascontiguousarray(xc),
            "mf": np.ascontiguousarray(mc.astype(np.float32)),
        })
    return in_maps


def kernel(input_data, target, mask, root_dir=0, _trace=False):
    nc = _get_nc()
    in_maps = _shard_inputs(np.asarray(input_data), np.asarray(target),
                            np.asarray(mask))
    res = run_bass_kernel_spmd(nc, in_maps, core_ids=list(range(NCORES)),
                               trace=_trace)
    outs = res.results
    val_sum = 0.0
    dot_sum = 0.0
    for om in outs:
        o = om["out"].astype(np.float64)
        val_sum += o[:, 0::2].sum()
        dot_sum += o[:, 1::2].sum()
    cnt = float((np.asarray(mask)[:, 0, 0] == 1).sum())
    val_loss = np.float32(val_sum / (3.0 * cnt))
    vec_loss = np.float32(1.0 - dot_sum / cnt)
    if _trace:
        return (val_loss, vec_loss), res
    return (val_loss, vec_loss)


# revision 12
# speedup vs baseline: 1.2713x; 1.2713x over previous
"""Trainium2 Bass kernel for the eigenvalue/eigenvector loss
(nn_AV_loss): per-voxel 3x3 symmetric eigendecomposition of input and
target tensors, masked L1 of sorted eigenvalues + masked principal-axis
|cosine|, reduced to two scalars.

Self-contained: hardcodes shapes/sharding. kernel(**inputs) takes FULL
inputs and returns the full output (val_loss, vec_loss).

Sharding: fully data-parallel over B*H (2*80 = 160 -> 20 H-slices per
core); per-core partial masked sums are returned and reduced on host.

Math (per 3x3 symmetric matrix A = [[a,d,e],[d,b,f],[e,f,c]]):
  trigonometric (Smith) eigensolver:
    q = tr/3, p = sqrt(p2/6) with p2 = sum of squared deviator entries,
    r = det(A-qI)/(2 p^3) clamped to [-1,1];
    half-angle arctan keeps the ACT input in [-1, 1]:
      acos(r)/2 = pi/4 + atan((sm-sp)/(sm+sp)),
      sp = sqrt((1+r)/2), sm = sqrt((1-r)/2)
    lam_max = q + 2p*sin(pi/3 - 2at/3), lam_min = q - 2p*sin(pi/3 + 2at/3),
    lam_mid = 3q - lam_max - lam_min.
  principal eigenvector via cross product of the first two rows of
  (A - lam_max I)  (parallel-rows failure set has measure ~0 and its
  bounded error washes out in the 512k-voxel masked mean).

Precision: inputs are converted to bf16 on the host (halves DMA bytes);
the elementwise pipeline runs bf16 on the DVE (2x/4x perf modes) with
f32 for the reciprocal-seeded chains and the final accumulations.
Validated end-to-end error vs the fp64 reference is ~2e-4 relative.

Engine split: DVE tensor-tensor chains; ACT all squares (with free
scale folding: Square(sqrt(2)*x) = 2x^2), sqrt/arctan/sin/abs; GPSIMD a
slice of adds/muls. ACT table-set phases (sqrt -> trig -> sqrt) are
enforced with explicit ordering edges so the scheduler cannot thrash
table loads.
"""

import numpy as np
import ml_dtypes

import concourse.tile as tile
from concourse import mybir
from concourse.bacc import Bacc
from concourse.bass_utils import run_bass_kernel_spmd
from bass_rust import add_dep_helper

AF = mybir.ActivationFunctionType
OP = mybir.AluOpType
F32 = mybir.dt.float32
BF16 = mybir.dt.bfloat16

NCORES = 8
B, C, H, W, D = 2, 6, 80, 80, 80
HS = H // (NCORES // B)          # 20 h-slices per core
VPC = HS * W * D                 # 128000 voxels per core
P = 128
FV = VPC // P                    # 1000 voxel columns per partition
NCH = 2                          # chunks along the free dim
FC = FV // NCH                   # voxel cols per chunk (500)
PK = 2 * FC                      # packed cols per chunk: [input | target]

CLAMP = 1.0 - 3e-7
PI3 = float(np.pi / 3.0)
SQRT2 = float(np.sqrt(2.0))


def _build():
    nc = Bacc()
    x = nc.dram_tensor("x", [C, P, NCH, PK], BF16, kind="ExternalInput")
    mf = nc.dram_tensor("mf", [P, NCH, FC], F32, kind="ExternalInput")
    out = nc.dram_tensor("out", [P, 2 * NCH], F32, kind="ExternalOutput")

    with tile.TileContext(nc) as tc:
        with tc.tile_pool(name="main", bufs=1) as pool:

            def T(tag, cols=PK, dt=BF16):     # per-chunk persisted value
                return pool.tile([P, cols], dt, tag=tag, bufs=NCH, name=tag)

            def TA():                          # phase-A bf16 transient
                return pool.tile([P, PK], BF16, tag="tA", bufs=12, name="tA")

            def TA32():                        # phase-A f32 transient
                return pool.tile([P, PK], F32, tag="tA32", bufs=6, name="tA32")

            def TB():                          # phase-B bf16 transient
                return pool.tile([P, PK], BF16, tag="tB", bufs=12, name="tB")

            def THB():                         # half-width bf16 transient
                return pool.tile([P, FC], BF16, tag="tHB", bufs=10, name="tHB")

            def TH32():                        # half-width f32 transient
                return pool.tile([P, FC], F32, tag="tH32", bufs=6, name="tH32")

            out_sb = pool.tile([P, 2 * NCH], F32, tag="out_sb", bufs=1)
            c05 = pool.tile([P, 1], F32, tag="c05", bufs=1)
            nc.vector.memset(c05, 0.5)
            pi3c = pool.tile([P, 1], F32, tag="pi3c", bufs=1)
            nc.vector.memset(pi3c, PI3)

            # ---- loads ----
            chans = []
            masks = []
            for cidx in range(NCH):
                cd = {}
                # _SYM_IDX packing: a=ch0 b=ch3 c=ch5 d=ch1 e=ch2 f=ch4
                for nm, ch in (("a", 0), ("b", 3), ("c", 5),
                               ("d", 1), ("e", 2), ("f", 4)):
                    tl = T("ch_" + nm)
                    nc.sync.dma_start(out=tl, in_=x[ch, :, cidx, :])
                    cd[nm] = tl
                mt = T("mf", cols=FC, dt=F32)
                nc.sync.dma_start(out=mt, in_=mf[:, cidx, :])
                chans.append(cd)
                masks.append(mt)

            acts_A = []   # ACT instructions per phase, for ordering edges
            acts_B = []
            acts_C = []

            # ---- phase A (sqrt act-set): invariants, p, r, atan arg ----
            pers = []
            for cidx in range(NCH):
                ch = chans[cidx]
                a, b, c = ch["a"], ch["b"], ch["c"]
                d, e, f = ch["d"], ch["e"], ch["f"]

                sAB = TA()
                nc.vector.tensor_add(out=sAB, in0=a, in1=b)
                s3 = T("s3")
                nc.vector.tensor_add(out=s3, in0=sAB, in1=c)
                q = T("q")
                nc.vector.tensor_scalar_mul(out=q, in0=s3, scalar1=1.0 / 3.0)
                aq = TA()
                nc.vector.tensor_sub(out=aq, in0=a, in1=q)
                bq = TA()
                nc.vector.tensor_sub(out=bq, in0=b, in1=q)
                cq = TA()
                nc.vector.tensor_sub(out=cq, in0=c, in1=q)
                # pre-doubled squares: Square(sqrt(2)x) = 2x^2 (free scale)
                dd2 = T("dd2")
                acts_A.append(nc.scalar.activation(
                    out=dd2, in_=d, func=AF.Square, scale=SQRT2).ins)
                ee2 = TA()
                acts_A.append(nc.scalar.activation(
                    out=ee2, in_=e, func=AF.Square, scale=SQRT2).ins)
                ff2 = TA()
                acts_A.append(nc.scalar.activation(
                    out=ff2, in_=f, func=AF.Square, scale=SQRT2).ins)
                aq2 = TA()
                acts_A.append(nc.scalar.activation(
                    out=aq2, in_=aq, func=AF.Square).ins)
                bq2 = TA()
                acts_A.append(nc.scalar.activation(
                    out=bq2, in_=bq, func=AF.Square).ins)
                cq2 = TA()
                acts_A.append(nc.scalar.activation(
                    out=cq2, in_=cq, func=AF.Square).ins)
                de = T("de")
                nc.gpsimd.tensor_tensor(out=de, in0=d, in1=e, op=OP.mult)
                p12 = TA()
                nc.gpsimd.tensor_tensor(out=p12, in0=dd2, in1=ee2, op=OP.add)
                p12b = TA()
                nc.vector.tensor_add(out=p12b, in0=p12, in1=ff2)
                t = TA()
                nc.gpsimd.tensor_tensor(out=t, in0=aq2, in1=bq2, op=OP.add)
                t2 = TA()
                nc.vector.tensor_add(out=t2, in0=t, in1=cq2)
                p2 = TA()
                nc.vector.tensor_add(out=p2, in0=t2, in1=p12b)
                # tp = 2p = sqrt(p2 * 2/3)
                tp = T("tp")
                acts_A.append(nc.scalar.activation(
                    out=tp, in_=p2, func=AF.Sqrt, scale=2.0 / 3.0).ins)
                tpsq = TA()
                acts_A.append(nc.scalar.activation(
                    out=tpsq, in_=tp, func=AF.Square).ins)       # 4p^2
                p3x = TA32()
                nc.vector.tensor_mul(out=p3x, in0=tpsq, in1=tp)  # 8p^3
                ip8 = TA32()
                nc.vector.reciprocal_approx_fast(out=ip8, in_=p3x)

                # 2*det = abc2 + def4 - aff - bee - cdd  (all pre-doubled)
                f4 = TA()
                nc.vector.tensor_scalar_mul(out=f4, in0=f, scalar1=4.0)
                def4 = TA()
                nc.vector.tensor_mul(out=def4, in0=de, in1=f4)
                cq2x = TA()
                nc.vector.tensor_scalar_mul(out=cq2x, in0=cq, scalar1=2.0)
                bc2 = TA()
                nc.vector.tensor_mul(out=bc2, in0=bq, in1=cq2x)
                abc2 = TA()
                nc.vector.tensor_mul(out=abc2, in0=aq, in1=bc2)
                aff = TA()
                nc.gpsimd.tensor_tensor(out=aff, in0=aq, in1=ff2, op=OP.mult)
                bee = TA()
                nc.gpsimd.tensor_tensor(out=bee, in0=bq, in1=ee2, op=OP.mult)
                cdd = TA()
                nc.vector.tensor_mul(out=cdd, in0=cq, in1=dd2)
                s1 = TA()
                nc.vector.tensor_add(out=s1, in0=abc2, in1=def4)
                s2d = TA()
                nc.gpsimd.tensor_tensor(out=s2d, in0=aff, in1=bee, op=OP.add)
                s3d = TA()
                nc.vector.tensor_add(out=s3d, in0=s2d, in1=cdd)
                D2 = TA()
                nc.vector.tensor_sub(out=D2, in0=s1, in1=s3d)

                # r = det/(2p^3) = (D2 * ip8) * 2, clamped to +-CLAMP
                r0 = TA()
                nc.vector.tensor_mul(out=r0, in0=D2, in1=ip8)
                rA = TA()
                nc.vector.tensor_scalar(out=rA, in0=r0, scalar1=2.0,
                                        scalar2=CLAMP, op0=OP.mult, op1=OP.min)
                r = TA()
                nc.vector.tensor_scalar_max(out=r, in0=rA, scalar1=-CLAMP)
                sp = TA32()
                acts_A.append(nc.scalar.activation(
                    out=sp, in_=r, func=AF.Sqrt, scale=0.5, bias=c05).ins)
                sm = TA32()
                acts_A.append(nc.scalar.activation(
                    out=sm, in_=r, func=AF.Sqrt, scale=-0.5, bias=c05).ins)
                num = TA32()
                nc.vector.tensor_sub(out=num, in0=sm, in1=sp)
                den = TA32()
                nc.gpsimd.tensor_tensor(out=den, in0=sm, in1=sp, op=OP.add)
                iden = TA32()
                nc.vector.reciprocal_approx_fast(out=iden, in_=den)
                arg = T("arg", dt=F32)
                nc.vector.tensor_mul(out=arg, in0=num, in1=iden)
                pers.append(dict(s3=s3, q=q, tp=tp, arg=arg, dd2=dd2, de=de))

            # ---- phase B (trig act-set) ----
            persB = []
            for cidx in range(NCH):
                ch = chans[cidx]
                pr = pers[cidx]
                a, b, d, e, f = ch["a"], ch["b"], ch["d"], ch["e"], ch["f"]
                s3, q, tp, arg = pr["s3"], pr["q"], pr["tp"], pr["arg"]
                dd2, de = pr["dd2"], pr["de"]

                at = TB()
                acts_B.append(nc.scalar.activation(
                    out=at, in_=arg, func=AF.Arctan).ins)
                c1 = TB()
                acts_B.append(nc.scalar.activation(
                    out=c1, in_=at, func=AF.Sin,
                    scale=-2.0 / 3.0, bias=pi3c).ins)
                nc3 = TB()
                acts_B.append(nc.scalar.activation(
                    out=nc3, in_=at, func=AF.Sin,
                    scale=2.0 / 3.0, bias=pi3c).ins)
                pc1 = TB()
                nc.vector.tensor_mul(out=pc1, in0=tp, in1=c1)
                l1 = TB()
                nc.vector.tensor_add(out=l1, in0=pc1, in1=q)    # lam_max
                pc3 = TB()
                nc.vector.tensor_mul(out=pc3, in0=tp, in1=nc3)
                l3 = TB()
                nc.vector.tensor_sub(out=l3, in0=q, in1=pc3)    # lam_min
                sl = TB()
                nc.gpsimd.tensor_tensor(out=sl, in0=l1, in1=l3, op=OP.add)
                l2 = TB()
                nc.vector.tensor_sub(out=l2, in0=s3, in1=sl)    # lam_mid

                # eigvec: cross(rows 0,1) of (A - l1*I)
                a1 = TB()
                nc.vector.tensor_sub(out=a1, in0=a, in1=l1)
                b1 = TB()
                nc.vector.tensor_sub(out=b1, in0=b, in1=l1)
                m1 = TB()
                nc.gpsimd.tensor_tensor(out=m1, in0=d, in1=f, op=OP.mult)
                m2 = TB()
                nc.vector.tensor_mul(out=m2, in0=e, in1=b1)
                w1 = TB()
                nc.vector.tensor_sub(out=w1, in0=m1, in1=m2)
                m4 = TB()
                nc.vector.tensor_mul(out=m4, in0=a1, in1=f)
                w2 = TB()
                nc.vector.tensor_sub(out=w2, in0=de, in1=m4)
                m5 = TB()
                nc.vector.tensor_mul(out=m5, in0=a1, in1=b1)
                dd05 = TB()
                nc.vector.tensor_scalar_mul(out=dd05, in0=dd2, scalar1=0.5)
                w3 = TB()
                nc.vector.tensor_sub(out=w3, in0=m5, in1=dd05)

                sw1 = TB()
                acts_B.append(nc.scalar.activation(
                    out=sw1, in_=w1, func=AF.Square).ins)
                sw2 = TB()
                acts_B.append(nc.scalar.activation(
                    out=sw2, in_=w2, func=AF.Square).ins)
                sw3 = TB()
                acts_B.append(nc.scalar.activation(
                    out=sw3, in_=w3, func=AF.Square).ins)
                n12 = TB()
                nc.gpsimd.tensor_tensor(out=n12, in0=sw1, in1=sw2, op=OP.add)
                nrm = TB()
                nc.vector.tensor_add(out=nrm, in0=n12, in1=sw3)

                def IH(tl):
                    return tl[:, 0:FC]

                def THF(tl):
                    return tl[:, FC:PK]

                nn0 = TH32()
                nc.vector.tensor_mul(out=nn0, in0=IH(nrm), in1=THF(nrm))
                nn = TH32()
                nc.vector.tensor_scalar_add(out=nn, in0=nn0, scalar1=1e-30)
                inn = T("inn", cols=FC, dt=F32)
                nc.vector.reciprocal_approx_fast(out=inn, in_=nn)

                d1 = THB()
                nc.vector.tensor_mul(out=d1, in0=IH(w1), in1=THF(w1))
                d2 = THB()
                nc.vector.tensor_mul(out=d2, in0=IH(w2), in1=THF(w2))
                d3 = THB()
                nc.vector.tensor_mul(out=d3, in0=IH(w3), in1=THF(w3))
                d12 = THB()
                nc.gpsimd.tensor_tensor(out=d12, in0=d1, in1=d2, op=OP.add)
                dotv = THB()
                nc.vector.tensor_add(out=dotv, in0=d12, in1=d3)
                adot = T("adot", cols=FC)
                acts_B.append(nc.scalar.activation(
                    out=adot, in_=dotv, func=AF.Abs).ins)

                dl1 = THB()
                nc.vector.tensor_sub(out=dl1, in0=IH(l1), in1=THF(l1))
                dl2 = THB()
                nc.vector.tensor_sub(out=dl2, in0=IH(l2), in1=THF(l2))
                dl3 = THB()
                nc.vector.tensor_sub(out=dl3, in0=IH(l3), in1=THF(l3))
                ab1 = THB()
                acts_B.append(nc.scalar.activation(
                    out=ab1, in_=dl1, func=AF.Abs).ins)
                ab2 = THB()
                acts_B.append(nc.scalar.activation(
                    out=ab2, in_=dl2, func=AF.Abs).ins)
                ab3 = THB()
                acts_B.append(nc.scalar.activation(
                    out=ab3, in_=dl3, func=AF.Abs).ins)
                s12 = THB()
                nc.gpsimd.tensor_tensor(out=s12, in0=ab1, in1=ab2, op=OP.add)
                sld = T("sld", cols=FC, dt=F32)
                nc.vector.tensor_add(out=sld, in0=s12, in1=ab3)
                persB.append(dict(inn=inn, adot=adot, sld=sld))

            # ---- phase C (sqrt act-set): normalize + masked reductions ----
            for cidx in range(NCH):
                pb = persB[cidx]
                mt = masks[cidx]
                rn = TH32()
                acts_C.append(nc.scalar.activation(
                    out=rn, in_=pb["inn"], func=AF.Sqrt).ins)
                dotn = TH32()
                nc.vector.tensor_mul(out=dotn, in0=pb["adot"], in1=rn)
                junk = TH32()
                nc.vector.scalar_tensor_tensor(
                    out=junk, in0=pb["sld"], scalar=1.0, in1=mt,
                    op0=OP.mult, op1=OP.mult,
                    accum_out=out_sb[:, 2 * cidx:2 * cidx + 1])
                junk2 = TH32()
                nc.vector.scalar_tensor_tensor(
                    out=junk2, in0=dotn, scalar=1.0, in1=mt,
                    op0=OP.mult, op1=OP.mult,
                    accum_out=out_sb[:, 2 * cidx + 1:2 * cidx + 2])

            nc.sync.dma_start(out=out[:, :], in_=out_sb)

            # ACT phase-ordering edges: all sqrt-set ops before any trig-set
            # op, all trig-set ops before the final sqrt-set ops. Ordering
            # edges only (same engine), so no extra semaphores.
            for later in acts_B:
                add_dep_helper(later, acts_A[-1], False,
                               "act table phase order A->B")
            for later in acts_C:
                add_dep_helper(later, acts_B[-1], False,
                               "act table phase order B->C")
    nc.finalize()
    return nc


_NC = None


def _get_nc():
    global _NC
    if _NC is None:
        _NC = _build()
    return _NC


def _shard_inputs(input_data, target, mask):
    """Full inputs -> per-core in_maps (host-side pack, f32 -> bf16)."""
    x = np.asarray(input_data, dtype=np.float32)
    t = np.asarray(target, dtype=np.float32)
    m = np.asarray(mask)
    in_maps = []
    for k in range(NCORES):
        bidx = k // (NCORES // B)
        h0 = HS * (k % (NCORES // B))
        xs = x[bidx, :, h0:h0 + HS].reshape(C, P, NCH, FC)
        ts = t[bidx, :, h0:h0 + HS].reshape(C, P, NCH, FC)
        xc = np.concatenate([xs, ts], axis=-1)          # [C,P,NCH,PK]
        mc = (m[bidx, 0, 0, h0:h0 + HS].reshape(P, NCH, FC) == 1)
        in_maps.append({
            "x": np.ascontiguousarray(xc.astype(ml_dtypes.bfloat16)),
            "mf": np.ascontiguousarray(mc.astype(np.float32)),
        })
    return in_maps


def kernel(input_data, target, mask, root_dir=0, _trace=False):
    nc = _get_nc()
    in_maps = _shard_inputs(np.asarray(input_data), np.asarray(target),
                            np.asarray(mask))
    res = run_bass_kernel_spmd(nc, in_maps, core_ids=list(range(NCORES)),
                               trace=_trace)
    outs = res.results
    val_sum = 0.0
    dot_sum = 0.0
    for om in outs:
        o = om["out"].astype(np.float64)
        val_sum += o[:, 0::2].sum()
        dot_sum += o[:, 1::2].sum()
    cnt = float((np.asarray(mask)[:, 0, 0] == 1).sum())
    val_loss = np.float32(val_sum / (3.0 * cnt))
    vec_loss = np.float32(1.0 - dot_sum / cnt)
    if _trace:
        return (val_loss, vec_loss), res
    return (val_loss, vec_loss)


# revision 13
# speedup vs baseline: 1.3499x; 1.0619x over previous
"""Trainium2 Bass kernel for the eigenvalue/eigenvector loss
(nn_AV_loss): per-voxel 3x3 symmetric eigendecomposition of input and
target tensors, masked L1 of sorted eigenvalues + masked principal-axis
|cosine|, reduced to two scalars.

Self-contained: hardcodes shapes/sharding. kernel(**inputs) takes FULL
inputs and returns the full output (val_loss, vec_loss).

Sharding: fully data-parallel over B*H (2*80 = 160 -> 20 H-slices per
core); per-core partial masked sums are returned and reduced on host.

Math (per 3x3 symmetric matrix A = [[a,d,e],[d,b,f],[e,f,c]]):
  trigonometric (Smith) eigensolver:
    q = tr/3, p = sqrt(p2/6) with p2 = sum of squared deviator entries,
    r = det(A-qI)/(2 p^3) clamped to [-1,1];
    half-angle arctan keeps the ACT input in [-1, 1]:
      acos(r)/2 = pi/4 + atan((sm-sp)/(sm+sp)),
      sp = sqrt((1+r)/2), sm = sqrt((1-r)/2)
    lam_max = q + 2p*sin(pi/3 - 2at/3), lam_min = q - 2p*sin(pi/3 + 2at/3),
    lam_mid = 3q - lam_max - lam_min.
  principal eigenvector via cross product of the first two rows of
  (A - lam_max I)  (parallel-rows failure set has measure ~0 and its
  bounded error washes out in the 512k-voxel masked mean).

Precision: inputs are converted to bf16 on the host (halves DMA bytes);
the elementwise pipeline runs bf16 on the DVE (2x/4x perf modes) with
f32 for the reciprocal-seeded chains and the final accumulations.
Validated end-to-end error vs the fp64 reference is ~2e-4 relative.

Engine split: DVE tensor-tensor chains; ACT all squares (with free
scale folding: Square(sqrt(2)*x) = 2x^2), sqrt/arctan/sin/abs; GPSIMD a
slice of adds/muls. ACT table-set phases (sqrt -> trig -> sqrt) are
enforced with explicit ordering edges so the scheduler cannot thrash
table loads.
"""

import numpy as np
import ml_dtypes

import concourse.tile as tile
from concourse import mybir
from concourse.bacc import Bacc
from concourse.bass_utils import run_bass_kernel_spmd
from bass_rust import add_dep_helper

AF = mybir.ActivationFunctionType
OP = mybir.AluOpType
F32 = mybir.dt.float32
BF16 = mybir.dt.bfloat16

NCORES = 8
B, C, H, W, D = 2, 6, 80, 80, 80
HS = H // (NCORES // B)          # 20 h-slices per core
VPC = HS * W * D                 # 128000 voxels per core
P = 128
FV = VPC // P                    # 1000 voxel columns per partition
NCH = 2                          # chunks along the free dim
FC = FV // NCH                   # voxel cols per chunk (500)
PK = 2 * FC                      # packed cols per chunk: [input | target]

CLAMP = 1.0 - 3e-7
PI3 = float(np.pi / 3.0)
SQRT2 = float(np.sqrt(2.0))


def _build():
    nc = Bacc()
    x = nc.dram_tensor("x", [C, P, NCH, PK], BF16, kind="ExternalInput")
    mf = nc.dram_tensor("mf", [P, NCH, FC], F32, kind="ExternalInput")
    out = nc.dram_tensor("out", [P, 2 * NCH], F32, kind="ExternalOutput")

    with tile.TileContext(nc) as tc:
        with tc.tile_pool(name="main", bufs=1) as pool:

            def T(tag, cols=PK, dt=BF16):     # per-chunk persisted value
                return pool.tile([P, cols], dt, tag=tag, bufs=NCH, name=tag)

            def TA():                          # phase-A bf16 transient
                return pool.tile([P, PK], BF16, tag="tA", bufs=12, name="tA")

            def TA32():                        # phase-A f32 transient
                return pool.tile([P, PK], F32, tag="tA32", bufs=6, name="tA32")

            def TB():                          # phase-B bf16 transient
                return pool.tile([P, PK], BF16, tag="tB", bufs=12, name="tB")

            def THB():                         # half-width bf16 transient
                return pool.tile([P, FC], BF16, tag="tHB", bufs=10, name="tHB")

            def TH32():                        # half-width f32 transient
                return pool.tile([P, FC], F32, tag="tH32", bufs=6, name="tH32")

            out_sb = pool.tile([P, 2 * NCH], F32, tag="out_sb", bufs=1)
            c05 = pool.tile([P, 1], F32, tag="c05", bufs=1)
            nc.vector.memset(c05, 0.5)
            pi3c = pool.tile([P, 1], F32, tag="pi3c", bufs=1)
            nc.vector.memset(pi3c, PI3)

            # ---- loads ----
            chans = []
            masks = []
            for cidx in range(NCH):
                cd = {}
                # _SYM_IDX packing: a=ch0 b=ch3 c=ch5 d=ch1 e=ch2 f=ch4
                for nm, ch in (("a", 0), ("b", 3), ("c", 5),
                               ("d", 1), ("e", 2), ("f", 4)):
                    tl = T("ch_" + nm)
                    nc.sync.dma_start(out=tl, in_=x[ch, :, cidx, :])
                    cd[nm] = tl
                mt = T("mf", cols=FC, dt=F32)
                nc.sync.dma_start(out=mt, in_=mf[:, cidx, :])
                chans.append(cd)
                masks.append(mt)

            acts_A = []   # ACT instructions per phase, for ordering edges
            acts_B = []
            acts_C = []

            # ---- phase A (sqrt act-set): invariants, p, r, atan arg ----
            pers = []
            for cidx in range(NCH):
                ch = chans[cidx]
                a, b, c = ch["a"], ch["b"], ch["c"]
                d, e, f = ch["d"], ch["e"], ch["f"]

                sAB = TA()
                nc.vector.tensor_add(out=sAB, in0=a, in1=b)
                s3 = T("s3")
                nc.vector.tensor_add(out=s3, in0=sAB, in1=c)
                q = T("q")
                nc.vector.tensor_scalar_mul(out=q, in0=s3, scalar1=1.0 / 3.0)
                aq = TA()
                nc.vector.tensor_sub(out=aq, in0=a, in1=q)
                bq = TA()
                nc.vector.tensor_sub(out=bq, in0=b, in1=q)
                cq = TA()
                nc.vector.tensor_sub(out=cq, in0=c, in1=q)
                # pre-doubled squares: Square(sqrt(2)x) = 2x^2 (free scale)
                dd2 = T("dd2")
                acts_A.append(nc.scalar.activation(
                    out=dd2, in_=d, func=AF.Square, scale=SQRT2).ins)
                ee2 = TA()
                acts_A.append(nc.scalar.activation(
                    out=ee2, in_=e, func=AF.Square, scale=SQRT2).ins)
                ff2 = TA()
                acts_A.append(nc.scalar.activation(
                    out=ff2, in_=f, func=AF.Square, scale=SQRT2).ins)
                aq2 = TA()
                acts_A.append(nc.scalar.activation(
                    out=aq2, in_=aq, func=AF.Square).ins)
                bq2 = TA()
                acts_A.append(nc.scalar.activation(
                    out=bq2, in_=bq, func=AF.Square).ins)
                cq2 = TA()
                acts_A.append(nc.scalar.activation(
                    out=cq2, in_=cq, func=AF.Square).ins)
                de = T("de")
                nc.gpsimd.tensor_tensor(out=de, in0=d, in1=e, op=OP.mult)
                p12 = TA()
                nc.vector.tensor_add(out=p12, in0=dd2, in1=ee2)
                p12b = TA()
                nc.vector.tensor_add(out=p12b, in0=p12, in1=ff2)
                t = TA()
                nc.vector.tensor_add(out=t, in0=aq2, in1=bq2)
                t2 = TA()
                nc.vector.tensor_add(out=t2, in0=t, in1=cq2)
                p2 = TA()
                nc.vector.tensor_add(out=p2, in0=t2, in1=p12b)
                # tp = 2p = sqrt(p2 * 2/3)
                tp = T("tp")
                acts_A.append(nc.scalar.activation(
                    out=tp, in_=p2, func=AF.Sqrt, scale=2.0 / 3.0).ins)
                tpsq = TA()
                acts_A.append(nc.scalar.activation(
                    out=tpsq, in_=tp, func=AF.Square).ins)       # 4p^2
                p3x = TA32()
                nc.vector.tensor_mul(out=p3x, in0=tpsq, in1=tp)  # 8p^3
                ip8 = TA32()
                nc.vector.reciprocal_approx_fast(out=ip8, in_=p3x)

                # 2*det = abc2 + def4 - aff - bee - cdd  (all pre-doubled)
                f4 = TA()
                nc.vector.tensor_scalar_mul(out=f4, in0=f, scalar1=4.0)
                def4 = TA()
                nc.vector.tensor_mul(out=def4, in0=de, in1=f4)
                cq2x = TA()
                nc.vector.tensor_scalar_mul(out=cq2x, in0=cq, scalar1=2.0)
                bc2 = TA()
                nc.vector.tensor_mul(out=bc2, in0=bq, in1=cq2x)
                abc2 = TA()
                nc.vector.tensor_mul(out=abc2, in0=aq, in1=bc2)
                aff = TA()
                nc.vector.tensor_mul(out=aff, in0=aq, in1=ff2)
                bee = TA()
                nc.vector.tensor_mul(out=bee, in0=bq, in1=ee2)
                cdd = TA()
                nc.vector.tensor_mul(out=cdd, in0=cq, in1=dd2)
                s1 = TA()
                nc.vector.tensor_add(out=s1, in0=abc2, in1=def4)
                s2d = TA()
                nc.vector.tensor_add(out=s2d, in0=aff, in1=bee)
                s3d = TA()
                nc.vector.tensor_add(out=s3d, in0=s2d, in1=cdd)
                D2 = TA()
                nc.vector.tensor_sub(out=D2, in0=s1, in1=s3d)

                # r = det/(2p^3) = (D2 * ip8) * 2, clamped to +-CLAMP
                r0 = TA()
                nc.vector.tensor_mul(out=r0, in0=D2, in1=ip8)
                rA = TA()
                nc.vector.tensor_scalar(out=rA, in0=r0, scalar1=2.0,
                                        scalar2=CLAMP, op0=OP.mult, op1=OP.min)
                r = TA()
                nc.vector.tensor_scalar_max(out=r, in0=rA, scalar1=-CLAMP)
                sp = TA32()
                acts_A.append(nc.scalar.activation(
                    out=sp, in_=r, func=AF.Sqrt, scale=0.5, bias=c05).ins)
                sm = TA32()
                acts_A.append(nc.scalar.activation(
                    out=sm, in_=r, func=AF.Sqrt, scale=-0.5, bias=c05).ins)
                num = TA32()
                nc.vector.tensor_sub(out=num, in0=sm, in1=sp)
                den = TA32()
                nc.vector.tensor_add(out=den, in0=sm, in1=sp)
                iden = TA32()
                nc.vector.reciprocal_approx_fast(out=iden, in_=den)
                arg = T("arg", dt=F32)
                nc.vector.tensor_mul(out=arg, in0=num, in1=iden)
                pers.append(dict(s3=s3, q=q, tp=tp, arg=arg, dd2=dd2, de=de))

            # ---- phase B (trig act-set) ----
            persB = []
            for cidx in range(NCH):
                ch = chans[cidx]
                pr = pers[cidx]
                a, b, d, e, f = ch["a"], ch["b"], ch["d"], ch["e"], ch["f"]
                s3, q, tp, arg = pr["s3"], pr["q"], pr["tp"], pr["arg"]
                dd2, de = pr["dd2"], pr["de"]

                at = TB()
                acts_B.append(nc.scalar.activation(
                    out=at, in_=arg, func=AF.Arctan).ins)
                c1 = TB()
                acts_B.append(nc.scalar.activation(
                    out=c1, in_=at, func=AF.Sin,
                    scale=-2.0 / 3.0, bias=pi3c).ins)
                nc3 = TB()
                acts_B.append(nc.scalar.activation(
                    out=nc3, in_=at, func=AF.Sin,
                    scale=2.0 / 3.0, bias=pi3c).ins)
                pc1 = TB()
                nc.vector.tensor_mul(out=pc1, in0=tp, in1=c1)
                l1 = TB()
                nc.vector.tensor_add(out=l1, in0=pc1, in1=q)    # lam_max
                pc3 = TB()
                nc.vector.tensor_mul(out=pc3, in0=tp, in1=nc3)
                l3 = TB()
                nc.vector.tensor_sub(out=l3, in0=q, in1=pc3)    # lam_min
                sl = TB()
                nc.vector.tensor_add(out=sl, in0=l1, in1=l3)
                l2 = TB()
                nc.vector.tensor_sub(out=l2, in0=s3, in1=sl)    # lam_mid

                # eigvec: cross(rows 0,1) of (A - l1*I)
                a1 = TB()
                nc.vector.tensor_sub(out=a1, in0=a, in1=l1)
                b1 = TB()
                nc.vector.tensor_sub(out=b1, in0=b, in1=l1)
                m1 = TB()
                nc.gpsimd.tensor_tensor(out=m1, in0=d, in1=f, op=OP.mult)
                m2 = TB()
                nc.vector.tensor_mul(out=m2, in0=e, in1=b1)
                w1 = TB()
                nc.vector.tensor_sub(out=w1, in0=m1, in1=m2)
                m4 = TB()
                nc.vector.tensor_mul(out=m4, in0=a1, in1=f)
                w2 = TB()
                nc.vector.tensor_sub(out=w2, in0=de, in1=m4)
                m5 = TB()
                nc.vector.tensor_mul(out=m5, in0=a1, in1=b1)
                dd05 = TB()
                nc.vector.tensor_scalar_mul(out=dd05, in0=dd2, scalar1=0.5)
                w3 = TB()
                nc.vector.tensor_sub(out=w3, in0=m5, in1=dd05)

                sw1 = TB()
                acts_B.append(nc.scalar.activation(
                    out=sw1, in_=w1, func=AF.Square).ins)
                sw2 = TB()
                acts_B.append(nc.scalar.activation(
                    out=sw2, in_=w2, func=AF.Square).ins)
                sw3 = TB()
                acts_B.append(nc.scalar.activation(
                    out=sw3, in_=w3, func=AF.Square).ins)
                n12 = TB()
                nc.vector.tensor_add(out=n12, in0=sw1, in1=sw2)
                nrm = TB()
                nc.vector.tensor_add(out=nrm, in0=n12, in1=sw3)

                def IH(tl):
                    return tl[:, 0:FC]

                def THF(tl):
                    return tl[:, FC:PK]

                nn0 = TH32()
                nc.gpsimd.tensor_tensor(out=nn0, in0=IH(nrm), in1=THF(nrm),
                                        op=OP.mult)
                nn = TH32()
                nc.vector.tensor_scalar_add(out=nn, in0=nn0, scalar1=1e-30)
                inn = T("inn", cols=FC, dt=F32)
                nc.vector.reciprocal_approx_fast(out=inn, in_=nn)

                d1 = THB()
                nc.gpsimd.tensor_tensor(out=d1, in0=IH(w1), in1=THF(w1),
                                        op=OP.mult)
                d2 = THB()
                nc.gpsimd.tensor_tensor(out=d2, in0=IH(w2), in1=THF(w2),
                                        op=OP.mult)
                d3 = THB()
                nc.gpsimd.tensor_tensor(out=d3, in0=IH(w3), in1=THF(w3),
                                        op=OP.mult)
                d12 = THB()
                nc.gpsimd.tensor_tensor(out=d12, in0=d1, in1=d2, op=OP.add)
                dotv = THB()
                nc.gpsimd.tensor_tensor(out=dotv, in0=d12, in1=d3, op=OP.add)
                adot = T("adot", cols=FC)
                acts_B.append(nc.scalar.activation(
                    out=adot, in_=dotv, func=AF.Abs).ins)

                dl1 = THB()
                nc.gpsimd.tensor_tensor(out=dl1, in0=IH(l1), in1=THF(l1),
                                        op=OP.subtract)
                dl2 = THB()
                nc.gpsimd.tensor_tensor(out=dl2, in0=IH(l2), in1=THF(l2),
                                        op=OP.subtract)
                dl3 = THB()
                nc.gpsimd.tensor_tensor(out=dl3, in0=IH(l3), in1=THF(l3),
                                        op=OP.subtract)
                ab1 = THB()
                acts_B.append(nc.scalar.activation(
                    out=ab1, in_=dl1, func=AF.Abs).ins)
                ab2 = THB()
                acts_B.append(nc.scalar.activation(
                    out=ab2, in_=dl2, func=AF.Abs).ins)
                ab3 = THB()
                acts_B.append(nc.scalar.activation(
                    out=ab3, in_=dl3, func=AF.Abs).ins)
                s12 = THB()
                nc.gpsimd.tensor_tensor(out=s12, in0=ab1, in1=ab2, op=OP.add)
                sld = T("sld", cols=FC, dt=F32)
                nc.gpsimd.tensor_tensor(out=sld, in0=s12, in1=ab3, op=OP.add)
                persB.append(dict(inn=inn, adot=adot, sld=sld))

            # ---- phase C (sqrt act-set): normalize + masked reductions ----
            for cidx in range(NCH):
                pb = persB[cidx]
                mt = masks[cidx]
                rn = TH32()
                acts_C.append(nc.scalar.activation(
                    out=rn, in_=pb["inn"], func=AF.Sqrt).ins)
                dotn = TH32()
                nc.gpsimd.tensor_tensor(out=dotn, in0=pb["adot"], in1=rn,
                                        op=OP.mult)
                junk = TH32()
                nc.vector.scalar_tensor_tensor(
                    out=junk, in0=pb["sld"], scalar=1.0, in1=mt,
                    op0=OP.mult, op1=OP.mult,
                    accum_out=out_sb[:, 2 * cidx:2 * cidx + 1])
                junk2 = TH32()
                nc.vector.scalar_tensor_tensor(
                    out=junk2, in0=dotn, scalar=1.0, in1=mt,
                    op0=OP.mult, op1=OP.mult,
                    accum_out=out_sb[:, 2 * cidx + 1:2 * cidx + 2])

            nc.sync.dma_start(out=out[:, :], in_=out_sb)

            # ACT phase-ordering edges: all sqrt-set ops before any trig-set
            # op, all trig-set ops before the final sqrt-set ops. Ordering
            # edges only (same engine), so no extra semaphores.
            for later in acts_B:
                add_dep_helper(later, acts_A[-1], False,
                               "act table phase order A->B")
            for later in acts_C:
                add_dep_helper(later, acts_B[-1], False,
                               "act table phase order B->C")
    nc.finalize()
    return nc


_NC = None


def _get_nc():
    global _NC
    if _NC is None:
        _NC = _build()
    return _NC


def _shard_inputs(input_data, target, mask):
    """Full inputs -> per-core in_maps (host-side pack, f32 -> bf16)."""
    x = np.asarray(input_data, dtype=np.float32)
    t = np.asarray(target, dtype=np.float32)
    m = np.asarray(mask)
    in_maps = []
    for k in range(NCORES):
        bidx = k // (NCORES // B)
        h0 = HS * (k % (NCORES // B))
        xs = x[bidx, :, h0:h0 + HS].reshape(C, P, NCH, FC)
        ts = t[bidx, :, h0:h0 + HS].reshape(C, P, NCH, FC)
        xc = np.concatenate([xs, ts], axis=-1)          # [C,P,NCH,PK]
        mc = (m[bidx, 0, 0, h0:h0 + HS].reshape(P, NCH, FC) == 1)
        in_maps.append({
            "x": np.ascontiguousarray(xc.astype(ml_dtypes.bfloat16)),
            "mf": np.ascontiguousarray(mc.astype(np.float32)),
        })
    return in_maps


def kernel(input_data, target, mask, root_dir=0, _trace=False):
    nc = _get_nc()
    in_maps = _shard_inputs(np.asarray(input_data), np.asarray(target),
                            np.asarray(mask))
    res = run_bass_kernel_spmd(nc, in_maps, core_ids=list(range(NCORES)),
                               trace=_trace)
    outs = res.results
    val_sum = 0.0
    dot_sum = 0.0
    for om in outs:
        o = om["out"].astype(np.float64)
        val_sum += o[:, 0::2].sum()
        dot_sum += o[:, 1::2].sum()
    cnt = float((np.asarray(mask)[:, 0, 0] == 1).sum())
    val_loss = np.float32(val_sum / (3.0 * cnt))
    vec_loss = np.float32(1.0 - dot_sum / cnt)
    if _trace:
        return (val_loss, vec_loss), res
    return (val_loss, vec_loss)
